# revision 2
# baseline (speedup 1.0000x reference)
"""Trainium2 Bass kernel for nn_CombineModel_wo_net (histogram_binning).

Full inputs in, full output out. Internally: data-parallel across 8
NeuronCores, 2 images per core. Each core streams its 2x3x544x960 fp32
slice from HBM (per-channel chunk DMAs on the SP HWDGE ring) and
reduces it to per-partition partials in an SBUF accumulator:
  col0: sum of s = c0+c1+c2 per pixel        (DVE, fused accum)
  col1: count(s >= 2.25)                     (DVE is_ge, bright pixels)
  col2: sum of sign((s - 0.75) * 2^124)      (ACT engine Sign)
The workload is HBM-bound (~305 GB/s/core with all 8 cores streaming);
the DVE does only 3 passes/pixel (t=c0+c1, s=t+c2, is_ge) and the ACT
engine does the second threshold, so both hide under the DMA stream.

Threshold equivalence: comparing s = c0+c1+c2 against 3*T is exact
w.r.t. the reference's g = mean(c) >= T because fp32 spacing at s~3T is
wider than the rounding interval of s/3 around T for T in {0.25, 0.75}.

Sign trick: count(s >= 0.75) = (sum(sign((s - 0.75) * 2^124)) + N) / 2.
scale = 2^124 and bias = -0.75 * 2^124 = -3*2^122 are exactly
representable; for s != 0.75 the scaled difference is >= 2^100 in
magnitude, far above the fp32 rounding error of s * 2^124 (~2^99), so
the sign is always correct. Only s == 0.75 exactly contributes 0
instead of +1, a <= 0.5-count error per hit (rel ~1e-6, harmless
against the 2e-2 gate and the ~3e-3 decision margins in the epilogue).

The tiny [5,16] epilogue (dynamic-range ratio, gap select, exposure
where-chains) is replicated exactly in float32 numpy on the host from
the gathered partials.
"""

import sys

for _p in ("/opt/trn_rl_repo",):
    if _p not in sys.path:
        sys.path.insert(0, _p)

from contextlib import ExitStack

import numpy as np

import concourse.bass as bass
import concourse.bacc as bacc
import concourse.mybir as mybir
import concourse.tile as tile
from concourse.bass_utils import run_bass_kernel_spmd

# Problem geometry (hardcoded per contract).
B, C, H, W = 16, 3, 544, 960
N_CORES = 8
IMGS_PER_CORE = B // N_CORES          # 2
PLANE = H * W                          # 522240 = 128 * 4080
P = 128
COLS = PLANE // P                      # 4080
NQ = 3                                 # sum_s, cnt_ge_2.25, sign-sum_0.75
# Per-image column splits. The last image tapers so that almost no
# compute remains after the final DMA byte arrives.
PLAN = [[2040, 2040], [2040, 1428, 408, 204]]
NACC = sum(len(p) for p in PLAN) * NQ  # 18 accumulator columns

F32 = mybir.dt.float32

# 2^124 and -0.75 * 2^124 are exactly representable in fp32.
SIGN_SCALE = float(2.0**124)
SIGN_BIAS_QUARTER = -float(0.75 * 2.0**124)

# Module-level knobs (test.py pokes these; grading path uses defaults).
TRACE = False
LAST_RESULT = None  # BassKernelResults of most recent run (for profiling)

_compiled_nc = None


def _build_bass(reps=1, plan=None, in_bufs=4, tmp_bufs=4, bits_bufs=4):
    """Emit the per-core Tile program (same SPMD program on all 8 cores).

    reps > 1 wraps the workload in a hardware For_i loop so one NEFF
    execution runs it `reps` times; the bench harness uses marginal
    time per iteration as the HW exec time. The grading path uses
    reps=1 (no loop).
    """
    if plan is None:
        plan = PLAN
    nacc = sum(len(p) for p in plan) * NQ
    nc = bacc.Bacc(
        "TRN2", target_bir_lowering=False, debug=False, num_devices=N_CORES
    )
    img = nc.dram_tensor(
        "img", [IMGS_PER_CORE, C, P, COLS], F32, kind="ExternalInput"
    ).ap()
    acc_out = nc.dram_tensor("acc", [P, nacc], F32, kind="ExternalOutput").ap()

    add = mybir.AluOpType.add
    is_ge = mybir.AluOpType.is_ge
    sign_f = mybir.ActivationFunctionType.Sign

    with ExitStack() as ctx:
        tc = ctx.enter_context(tile.TileContext(nc))
        pool_in = ctx.enter_context(tc.tile_pool(name="inp", bufs=in_bufs))
        pool_tmp = ctx.enter_context(tc.tile_pool(name="tmp", bufs=tmp_bufs))
        pool_bits = ctx.enter_context(tc.tile_pool(name="bitsp", bufs=bits_bufs))
        pool_acc = ctx.enter_context(tc.tile_pool(name="accsb", bufs=1))

        acc_sb = pool_acc.tile([P, nacc], F32, tag="acc")
        bias_q = pool_acc.tile([P, 1], F32, tag="bias_q")
        nc.vector.memset(bias_q[:], SIGN_BIAS_QUARTER)

        def workload():
            col = 0
            for i in range(IMGS_PER_CORE):
                start = 0
                for size in plan[i]:
                    sl = slice(start, start + size)
                    start += size
                    c0 = pool_in.tile([P, size], F32, tag="c0")
                    nc.sync.dma_start(c0[:], img[i, 0, :, sl])
                    c1 = pool_in.tile([P, size], F32, tag="c1")
                    nc.sync.dma_start(c1[:], img[i, 1, :, sl])
                    c2 = pool_in.tile([P, size], F32, tag="c2")
                    nc.sync.dma_start(c2[:], img[i, 2, :, sl])

                    t = pool_tmp.tile([P, size], F32, tag="t")
                    nc.vector.tensor_tensor(t[:], c0[:], c1[:], add)
                    # s = (t + 0.0) + c2, fused row-sum into acc column
                    s = pool_tmp.tile([P, size], F32, tag="s")
                    nc.vector.scalar_tensor_tensor(
                        s[:], t[:], 0.0, c2[:], add, add,
                        accum_out=acc_sb[:, col : col + 1],
                    )
                    # bright count on DVE (exact is_ge)
                    b1 = pool_bits.tile([P, size], F32, tag="bits")
                    nc.vector.tensor_scalar(
                        b1[:], s[:], 2.25, None, is_ge, add,
                        accum_out=acc_sb[:, col + 1 : col + 2],
                    )
                    # quarter-threshold sign-sum on the ACT engine
                    b2 = pool_bits.tile([P, size], F32, tag="bits")
                    nc.scalar.activation(
                        b2[:], s[:], sign_f,
                        bias=bias_q[:], scale=SIGN_SCALE,
                        accum_out=acc_sb[:, col + 2 : col + 3],
                    )
                    col += 3

        if reps == 1:
            workload()
        else:
            with tc.For_i(0, reps, 1):
                workload()

        nc.sync.dma_start(acc_out[:, :], acc_sb[:])

    nc.compile()
    return nc, nacc


def _get_nc():
    global _compiled_nc
    if _compiled_nc is None:
        _compiled_nc = _build_bass(plan=PLAN)[0]
    return _compiled_nc


def kernel(batch_images, base_exposure_1, base_exposure_2):
    global LAST_RESULT
    batch_images = np.ascontiguousarray(np.asarray(batch_images, dtype=np.float32))
    be1 = np.asarray(base_exposure_1, dtype=np.float32)
    be2 = np.asarray(base_exposure_2, dtype=np.float32)
    assert batch_images.shape == (B, C, H, W)

    nc = _get_nc()
    shards = batch_images.reshape(N_CORES, IMGS_PER_CORE, C, P, COLS)
    in_maps = [{"img": shards[c]} for c in range(N_CORES)]
    res = run_bass_kernel_spmd(nc, in_maps, list(range(N_CORES)), trace=TRACE)
    LAST_RESULT = res

    # ---- gather/unshard: fold per-partition partials to per-image stats ----
    sum_s = np.empty(B, dtype=np.float64)
    cnt_bright = np.empty(B, dtype=np.float64)
    cnt_ge_quarter = np.empty(B, dtype=np.float64)
    for c in range(N_CORES):
        acc = np.asarray(res.results[c]["acc"], dtype=np.float64)  # [128, NACC]
        col = 0
        for i, sizes in enumerate(PLAN):
            b = c * IMGS_PER_CORE + i
            s = b1 = b2 = 0.0
            for _ in sizes:
                s += acc[:, col].sum()
                b1 += acc[:, col + 1].sum()
                b2 += acc[:, col + 2].sum()
                col += 3
            sum_s[b] = s
            cnt_bright[b] = b1
            # col2 holds sum(sign(s - 0.75)): count = (sign_sum + N) / 2
            cnt_ge_quarter[b] = (b2 + PLANE) / 2.0

    # ---- epilogue: replicate reference numerics in fp32 ----
    f32 = np.float32
    bright = cnt_bright.astype(np.float32)                     # exact counts
    dark = (np.float64(PLANE) - cnt_ge_quarter).astype(np.float32)
    dr = bright / (dark + f32(1e-5))
    bright_avg = (sum_s / 3.0 / PLANE).astype(np.float32)

    g = f32(0.5)
    conds = [
        (dr > f32(1.0)) & (bright_avg > f32(0.4)) & (bright_avg < f32(0.6)),
        bright_avg <= f32(0.3),
        bright_avg >= f32(0.7),
        (dr <= f32(1.0)) & (bright_avg > f32(0.3)) & (bright_avg < f32(0.7)),
    ]
    vals = [g * f32(2.0), g * f32(0.5), g * f32(0.5), g * f32(0.75)]
    gaps = np.select(conds, vals, f32(0.0)).astype(np.float32)

    bl = bright_avg[-1]
    gl = gaps[-1]
    s_ = f32(1.7)
    e1 = np.where(
        bl <= f32(0.25), be1 + f32(0.5) * gl * s_,
        np.where(bl >= f32(0.75), be1 - f32(0.5) * gl * s_, be1 - f32(0.3) * gl),
    ).astype(np.float32)
    e2 = np.where(
        bl <= f32(0.25), be2 + f32(0.5) * gl * s_,
        np.where(bl >= f32(0.75), be2 - f32(0.5) * gl * s_, be2 + f32(0.7) * gl),
    ).astype(np.float32)

    return np.stack([dr, bright_avg, gaps, e1, e2]).astype(np.float32)


# revision 3
# speedup vs baseline: 1.0194x; 1.0194x over previous
"""Trainium2 Bass kernel for nn_CombineModel_wo_net (histogram_binning).

Full inputs in, full output out. Internally: data-parallel across 8
NeuronCores, 2 images per core. Each core streams its 2x3x544x960 fp32
slice from HBM (per-channel chunk DMAs on the SP HWDGE ring) and
reduces it to per-partition partials in an SBUF accumulator:
  col0: sum of s = c0+c1+c2 per pixel        (DVE, fused accum)
  col1: count(s >= 2.25)                     (DVE is_ge, bright pixels)
  col2: sum of sign((s - 0.75) * 2^124)      (ACT engine Sign)
The workload is HBM-bound (~305 GB/s/core with all 8 cores streaming);
the DVE does only 3 passes/pixel (t=c0+c1, s=t+c2, is_ge) and the ACT
engine does the second threshold, so both hide under the DMA stream.

Threshold equivalence: comparing s = c0+c1+c2 against 3*T is exact
w.r.t. the reference's g = mean(c) >= T because fp32 spacing at s~3T is
wider than the rounding interval of s/3 around T for T in {0.25, 0.75}.

Sign trick: count(s >= 0.75) = (sum(sign((s - 0.75) * 2^124)) + N) / 2.
scale = 2^124 and bias = -0.75 * 2^124 = -3*2^122 are exactly
representable; for s != 0.75 the scaled difference is >= 2^100 in
magnitude, far above the fp32 rounding error of s * 2^124 (~2^99), so
the sign is always correct. Only s == 0.75 exactly contributes 0
instead of +1, a <= 0.5-count error per hit (rel ~1e-6, harmless
against the 2e-2 gate and the ~3e-3 decision margins in the epilogue).

The tiny [5,16] epilogue (dynamic-range ratio, gap select, exposure
where-chains) is replicated exactly in float32 numpy on the host from
the gathered partials.
"""

import sys

for _p in ("/opt/trn_rl_repo",):
    if _p not in sys.path:
        sys.path.insert(0, _p)

from contextlib import ExitStack

import numpy as np

import concourse.bass as bass
import concourse.bacc as bacc
import concourse.mybir as mybir
import concourse.tile as tile
from concourse.bass_utils import run_bass_kernel_spmd

# Problem geometry (hardcoded per contract).
B, C, H, W = 16, 3, 544, 960
N_CORES = 8
IMGS_PER_CORE = B // N_CORES          # 2
PLANE = H * W                          # 522240 = 128 * 4080
P = 128
COLS = PLANE // P                      # 4080
NQ = 3                                 # sum_s, cnt_ge_2.25, sign-sum_0.75
# Per-image column splits. The last image tapers so that almost no
# compute remains after the final DMA byte arrives.
PLAN = [[2040, 2040], [2040, 1428, 408, 204]]
NACC = sum(len(p) for p in PLAN) * NQ  # 18 accumulator columns

F32 = mybir.dt.float32

# 2^124 and -0.75 * 2^124 are exactly representable in fp32.
SIGN_SCALE = float(2.0**124)
SIGN_BIAS_QUARTER = -float(0.75 * 2.0**124)

# Module-level knobs (test.py pokes these; grading path uses defaults).
TRACE = False
LAST_RESULT = None  # BassKernelResults of most recent run (for profiling)

_compiled_nc = None


def _build_bass(reps=1, plan=None, in_bufs=4, tmp_bufs=3, bits_bufs=3):
    """Emit the per-core Tile program (same SPMD program on all 8 cores).

    reps > 1 wraps the workload in a hardware For_i loop so one NEFF
    execution runs it `reps` times; the bench harness uses marginal
    time per iteration as the HW exec time. The grading path uses
    reps=1 (no loop).
    """
    if plan is None:
        plan = PLAN
    nacc = sum(len(p) for p in plan) * NQ
    nc = bacc.Bacc(
        "TRN2", target_bir_lowering=False, debug=False, num_devices=N_CORES
    )
    img = nc.dram_tensor(
        "img", [IMGS_PER_CORE, C, P, COLS], F32, kind="ExternalInput"
    ).ap()
    acc_out = nc.dram_tensor("acc", [P, nacc], F32, kind="ExternalOutput").ap()

    add = mybir.AluOpType.add
    is_ge = mybir.AluOpType.is_ge
    sign_f = mybir.ActivationFunctionType.Sign

    with ExitStack() as ctx:
        tc = ctx.enter_context(tile.TileContext(nc))
        pool_in = ctx.enter_context(tc.tile_pool(name="inp", bufs=in_bufs))
        pool_tmp = ctx.enter_context(tc.tile_pool(name="tmp", bufs=tmp_bufs))
        pool_bits = ctx.enter_context(tc.tile_pool(name="bitsp", bufs=bits_bufs))
        pool_acc = ctx.enter_context(tc.tile_pool(name="accsb", bufs=1))

        acc_sb = pool_acc.tile([P, nacc], F32, tag="acc")
        bias_q = pool_acc.tile([P, 1], F32, tag="bias_q")
        nc.vector.memset(bias_q[:], SIGN_BIAS_QUARTER)

        def workload():
            col = 0
            for i in range(IMGS_PER_CORE):
                start = 0
                for size in plan[i]:
                    sl = slice(start, start + size)
                    start += size
                    c0 = pool_in.tile([P, size], F32, tag="c0")
                    nc.sync.dma_start(c0[:], img[i, 0, :, sl])
                    c1 = pool_in.tile([P, size], F32, tag="c1")
                    nc.sync.dma_start(c1[:], img[i, 1, :, sl])
                    c2 = pool_in.tile([P, size], F32, tag="c2")
                    nc.sync.dma_start(c2[:], img[i, 2, :, sl])

                    t = pool_tmp.tile([P, size], F32, tag="t")
                    nc.vector.tensor_tensor(t[:], c0[:], c1[:], add)
                    # s = (t + 0.0) + c2, fused row-sum into acc column
                    s = pool_tmp.tile([P, size], F32, tag="s")
                    nc.vector.scalar_tensor_tensor(
                        s[:], t[:], 0.0, c2[:], add, add,
                        accum_out=acc_sb[:, col : col + 1],
                    )
                    # bright count on DVE (exact is_ge)
                    b1 = pool_bits.tile([P, size], F32, tag="bits")
                    nc.vector.tensor_scalar(
                        b1[:], s[:], 2.25, None, is_ge, add,
                        accum_out=acc_sb[:, col + 1 : col + 2],
                    )
                    # quarter-threshold sign-sum on the ACT engine
                    b2 = pool_bits.tile([P, size], F32, tag="bits")
                    nc.scalar.activation(
                        b2[:], s[:], sign_f,
                        bias=bias_q[:], scale=SIGN_SCALE,
                        accum_out=acc_sb[:, col + 2 : col + 3],
                    )
                    col += 3

        if reps == 1:
            workload()
        else:
            with tc.For_i(0, reps, 1):
                workload()

        nc.sync.dma_start(acc_out[:, :], acc_sb[:])

    nc.compile()
    return nc, nacc


def _get_nc():
    global _compiled_nc
    if _compiled_nc is None:
        _compiled_nc = _build_bass(plan=PLAN)[0]
    return _compiled_nc


def kernel(batch_images, base_exposure_1, base_exposure_2):
    global LAST_RESULT
    batch_images = np.ascontiguousarray(np.asarray(batch_images, dtype=np.float32))
    be1 = np.asarray(base_exposure_1, dtype=np.float32)
    be2 = np.asarray(base_exposure_2, dtype=np.float32)
    assert batch_images.shape == (B, C, H, W)

    nc = _get_nc()
    shards = batch_images.reshape(N_CORES, IMGS_PER_CORE, C, P, COLS)
    in_maps = [{"img": shards[c]} for c in range(N_CORES)]
    res = run_bass_kernel_spmd(nc, in_maps, list(range(N_CORES)), trace=TRACE)
    LAST_RESULT = res

    # ---- gather/unshard: fold per-partition partials to per-image stats ----
    sum_s = np.empty(B, dtype=np.float64)
    cnt_bright = np.empty(B, dtype=np.float64)
    cnt_ge_quarter = np.empty(B, dtype=np.float64)
    for c in range(N_CORES):
        acc = np.asarray(res.results[c]["acc"], dtype=np.float64)  # [128, NACC]
        col = 0
        for i, sizes in enumerate(PLAN):
            b = c * IMGS_PER_CORE + i
            s = b1 = b2 = 0.0
            for _ in sizes:
                s += acc[:, col].sum()
                b1 += acc[:, col + 1].sum()
                b2 += acc[:, col + 2].sum()
                col += 3
            sum_s[b] = s
            cnt_bright[b] = b1
            # col2 holds sum(sign(s - 0.75)): count = (sign_sum + N) / 2
            cnt_ge_quarter[b] = (b2 + PLANE) / 2.0

    # ---- epilogue: replicate reference numerics in fp32 ----
    f32 = np.float32
    bright = cnt_bright.astype(np.float32)                     # exact counts
    dark = (np.float64(PLANE) - cnt_ge_quarter).astype(np.float32)
    dr = bright / (dark + f32(1e-5))
    bright_avg = (sum_s / 3.0 / PLANE).astype(np.float32)

    g = f32(0.5)
    conds = [
        (dr > f32(1.0)) & (bright_avg > f32(0.4)) & (bright_avg < f32(0.6)),
        bright_avg <= f32(0.3),
        bright_avg >= f32(0.7),
        (dr <= f32(1.0)) & (bright_avg > f32(0.3)) & (bright_avg < f32(0.7)),
    ]
    vals = [g * f32(2.0), g * f32(0.5), g * f32(0.5), g * f32(0.75)]
    gaps = np.select(conds, vals, f32(0.0)).astype(np.float32)

    bl = bright_avg[-1]
    gl = gaps[-1]
    s_ = f32(1.7)
    e1 = np.where(
        bl <= f32(0.25), be1 + f32(0.5) * gl * s_,
        np.where(bl >= f32(0.75), be1 - f32(0.5) * gl * s_, be1 - f32(0.3) * gl),
    ).astype(np.float32)
    e2 = np.where(
        bl <= f32(0.25), be2 + f32(0.5) * gl * s_,
        np.where(bl >= f32(0.75), be2 - f32(0.5) * gl * s_, be2 + f32(0.7) * gl),
    ).astype(np.float32)

    return np.stack([dr, bright_avg, gaps, e1, e2]).astype(np.float32)
